# revision 38
# baseline (speedup 1.0000x reference)
"""Multi-head attention (RoPE + causal softmax) Trainium2 Bass kernel.

Problem: nn_MultiHeadAttention (B=16, S=512, D=1024, H=16, Hd=64).
Sharding: data-parallel over batch — 2 batches per core on 8 NeuronCores.

Device-side layout is feature-major ("transposed"): activations live as
[d, token] tiles so the d contraction sits on SBUF partitions for every
matmul. Per core:

  xT        [1024, 1024]  bf16   x shard, feature-major (col = b*512 + s)
  WqT/WkT/WvT/WoT [1024, 1024] bf16  (nn.Linear weight, transposed)
  cosT/tanT [128, 1024]   bf16   RoPE tables, replicated per 2-head chunk
  RT        [128, 128]    bf16   rotate_half as matrix (block-diag, transposed)
  mask2     [128, 256]    bf16   causal 0/1 mask for diagonal blocks, 2 heads
  outT      [1024, 1024]  fp32   output, feature-major

RoPE uses the tan trick: since the RoPE tables repeat with period 32
(emb = concat(freqs, freqs)), R@(q_bf16 * tan) * cos == rotate_half(q)*sin,
so the rotation matmul accumulates straight into the projection PSUM
(start=False continuation) and RoPE costs only 2 DVE ops (one all-bf16).
Causal mask is one bf16 [128,2,128] DVE mul per (pair, chunk) covering both
heads. Softmax denominators are batched per head-pair: the two PSUM ones-rows
DMA into a [128,8] tile, one reciprocal, DMA to a DRAM slot, and a stride-0
partition-broadcast DMA reads back [128,512] (h0 rows 0-63, h1 rows 64-127)
for two normalize muls. Wo results DMA from PSUM directly to DRAM.
Emission interleaves attention pairs with projection/wo groups so dense
N=512 matmul bursts fill attention's dependency gaps and the PE clock
gate (HAM) stays warm. Host reassembles [16, 512, 1024] fp32.
"""

import numpy as np
import ml_dtypes

BF16 = ml_dtypes.bfloat16

B, S, D = 16, 512, 1024
H, HD = 16, 64
NCORES = 8
BPC = B // NCORES          # batches per core
T = BPC * S                # tokens per core

_CACHE = {}


def _rope_tables():
    inv_freq = 1.0 / (10000.0 ** (np.arange(0, HD, 2, dtype=np.float64) / HD))
    t = np.arange(S, dtype=np.float64)
    freqs = np.outer(t, inv_freq)                    # [S, 32]
    emb = np.concatenate([freqs, freqs], -1)         # [S, 64]
    return np.cos(emb), np.sin(emb)                  # [S, 64] fp64


def _host_consts():
    cos, sin = _rope_tables()
    tan = sin / cos
    cols = np.arange(T) % S
    cosT = np.ascontiguousarray(np.tile(cos[cols].T, (2, 1))).astype(BF16)  # [128, T]
    tanT = np.ascontiguousarray(np.tile(tan[cols].T, (2, 1))).astype(BF16)
    R64 = np.zeros((64, 64), np.float32)
    R64[np.arange(32), np.arange(32) + 32] = -1.0
    R64[np.arange(32) + 32, np.arange(32)] = 1.0
    R128 = np.zeros((128, 128), np.float32)
    R128[:64, :64] = R64
    R128[64:, 64:] = R64
    RT = np.ascontiguousarray(R128.T).astype(BF16)
    mask01 = (np.arange(128)[None, :] >= np.arange(128)[:, None]).astype(BF16)  # [kt, qt]
    mask2 = np.ascontiguousarray(np.concatenate([mask01, mask01], axis=1))  # [128, 256]
    return cosT, tanT, RT, mask2


def _build_bass(dump_debug=False):
    import concourse.bacc as bacc
    import concourse.tile as tile
    import concourse.mybir as mybir

    dt = mybir.dt
    f32, bf16 = dt.float32, dt.bfloat16
    Exp = mybir.ActivationFunctionType.Exp

    nc = bacc.Bacc("TRN2", target_bir_lowering=False, debug=False, enable_asserts=False)

    xT_d = nc.dram_tensor("xT", [D, T], bf16, kind="ExternalInput").ap()
    wq_d = nc.dram_tensor("WqT", [D, D], bf16, kind="ExternalInput").ap()
    wk_d = nc.dram_tensor("WkT", [D, D], bf16, kind="ExternalInput").ap()
    wv_d = nc.dram_tensor("WvT", [D, D], bf16, kind="ExternalInput").ap()
    wo_d = nc.dram_tensor("WoT", [D, D], bf16, kind="ExternalInput").ap()
    cos_d = nc.dram_tensor("cosT", [128, T], bf16, kind="ExternalInput").ap()
    tan_d = nc.dram_tensor("tanT", [128, T], bf16, kind="ExternalInput").ap()
    rt_d = nc.dram_tensor("RT", [128, 128], bf16, kind="ExternalInput").ap()
    mask_d = nc.dram_tensor("mask2", [128, 256], bf16, kind="ExternalInput").ap()
    out_d = nc.dram_tensor("outT", [D, T], bf16, kind="ExternalOutput").ap()
    if dump_debug:
        qrot_d = nc.dram_tensor("qrotD", [D, T], bf16, kind="ExternalOutput").ap()
        krot_d = nc.dram_tensor("krotD", [D, T], bf16, kind="ExternalOutput").ap()
        v_d = nc.dram_tensor("vD", [T, H * 65], bf16, kind="ExternalOutput").ap()
        att_d = nc.dram_tensor("attD", [D, T], bf16, kind="ExternalOutput").ap()
        rb_d = nc.dram_tensor("rbD", [16, 128, S], f32, kind="ExternalOutput").ap()

    KC = D // 128  # 8 contraction chunks

    with tile.TileContext(nc) as tc:
        with (
            tc.tile_pool(name="consts", bufs=1) as consts,
            tc.tile_pool(name="persist", bufs=1) as persist,
            tc.tile_pool(name="work", bufs=3) as work,
            tc.tile_pool(name="expp", bufs=3) as expp,
            tc.tile_pool(name="ps_a", bufs=4, space="PSUM") as ps_a,
            tc.tile_pool(name="ps_b", bufs=2, space="PSUM") as ps_b,
            tc.tile_pool(name="dscr", bufs=4, space="DRAM") as dscr,
        ):
            # ---- resident input tiles (DMAs emitted in explicit order below)
            xT = [consts.tile([128, T], bf16, name=f"xT{k}") for k in range(KC)]
            wq = [consts.tile([128, D], bf16, name=f"wq{k}") for k in range(KC)]
            wk = [consts.tile([128, D], bf16, name=f"wk{k}") for k in range(KC)]
            wv = [consts.tile([128, D], bf16, name=f"wv{k}") for k in range(KC)]
            wo = [consts.tile([128, D], bf16, name=f"wo{k}") for k in range(KC)]
            RT = consts.tile([128, 128], bf16, name="RT")
            cosT = consts.tile([128, T], bf16, name="cosT")
            tanT = consts.tile([128, T], bf16, name="tanT")
            mask2 = consts.tile([128, 2, 128], bf16, name="mask2")

            def dma(dst, src):
                nc.sync.dma_start(out=dst, in_=src)

            # load order ~ first use: xT + q-proj m=0/1 slices + RoPE consts,
            # then k-proj slices + v(b0 heads 0-7) weights, then the rest.
            for k in range(KC):
                rows = slice(k * 128, (k + 1) * 128)
                dma(xT[k], xT_d[rows, :])
            for k in range(KC):
                rows = slice(k * 128, (k + 1) * 128)
                dma(wq[k][:, 0:128], wq_d[rows, 0:128])
            dma(RT, rt_d)
            dma(cosT, cos_d)
            dma(tanT, tan_d)
            for k in range(KC):
                rows = slice(k * 128, (k + 1) * 128)
                dma(wq[k][:, 128:256], wq_d[rows, 128:256])
            for k in range(KC):
                rows = slice(k * 128, (k + 1) * 128)
                dma(wk[k][:, 0:256], wk_d[rows, 0:256])
            for k in range(KC):
                rows = slice(k * 128, (k + 1) * 128)
                dma(wv[k][:, 0:S], wv_d[rows, 0:S])
            for k in range(KC):
                rows = slice(k * 128, (k + 1) * 128)
                dma(wq[k][:, 256:D], wq_d[rows, 256:D])
            for k in range(KC):
                rows = slice(k * 128, (k + 1) * 128)
                dma(wk[k][:, 256:D], wk_d[rows, 256:D])
            dma(mask2, mask_d)
            for k in range(KC):
                rows = slice(k * 128, (k + 1) * 128)
                dma(wv[k][:, S:D], wv_d[rows, S:D])
            for k in range(KC):
                rows = slice(k * 128, (k + 1) * 128)
                dma(wo[k], wo_d[rows, :])

            # ---- persistent intermediates
            qrot = [persist.tile([128, T], bf16, name=f"qrot{m}") for m in range(KC)]
            krot = [persist.tile([128, T], bf16, name=f"krot{m}") for m in range(KC)]
            # v token-major, per head padded with a ones column (65 per head)
            vsb = [persist.tile([128, H * 65], bf16, name=f"vsb{t_}") for t_ in range(T // 128)]
            att = [persist.tile([128, T], bf16, name=f"att{m}") for m in range(KC)]

            for t_ in range(T // 128):
                vt = vsb[t_].rearrange("p (h w) -> p h w", w=65)
                nc.gpsimd.memset(vt[:, :, 64:65], 1.0)

            # ---- phase emitters
            # qk groups are split into A (projection matmuls + ACT/DVE prep,
            # no PE dependency) and B (rotation matmul + rot mul) so the PE
            # never idles waiting for the pre->pre2 chain: B is emitted a
            # full PE burst after its A.
            def emit_qk_A(nb, w_sb, rot, m):
                cols = slice(nb * S, (nb + 1) * S)
                pp = ps_a.tile([128, S], f32, name="pp", tag="acc", bufs=2)
                for k in range(KC):
                    nc.tensor.matmul(
                        pp, w_sb[k][:, m * 128:(m + 1) * 128], xT[k][:, cols],
                        start=(k == 0), stop=(k == KC - 1))
                pre = work.tile([128, S], bf16, name="pre", tag="pre", bufs=2)
                nc.scalar.copy(pre, pp)              # ACT: psum -> sbuf bf16
                pre2 = work.tile([128, S], bf16, name="pre2", tag="pre2", bufs=2)
                nc.vector.tensor_mul(pre2, pre, tanT[:, cols])
                return (pp, pre2, rot, m, cols)

            def emit_qk_B(st):
                pp, pre2, rot, m, cols = st
                # rotation accumulates into the closed projection group:
                # pp += R @ (pre*tan); then rot = pp * cos == q*cos + rot_half(q)*sin
                nc.tensor.matmul(pp, RT, pre2, start=False, stop=True,
                                 skip_group_check=True)
                nc.vector.tensor_mul(rot[m][:, cols], pp, cosT[:, cols])

            def emit_v_group(tch, nh):
                # token-major v: x^T chunks as stationary operand
                # tch in 0..7 spans both batches (b = tch // 4)
                vt = vsb[tch].rearrange("p (h w) -> p h w", w=65)
                vp = ps_a.tile([128, S], f32, name="vp", tag="acc", bufs=2)
                for k in range(KC):
                    nc.tensor.matmul(
                        vp, xT[k][:, tch * 128:(tch + 1) * 128],
                        wv[k][:, nh * S:(nh + 1) * S],
                        start=(k == 0), stop=(k == KC - 1))
                # ACT copy into strided per-head layout (cast bf16)
                nc.scalar.copy(
                    vt[:, nh * 8:(nh + 1) * 8, 0:64],
                    vp.rearrange("p (h w) -> p h w", w=64))

            def emit_pair_scores(b, j):
                # head pair: the two K=64 score matmuls sit in disjoint PE
                # row-groups (partition bases 0/64), run concurrently, and land
                # in the two banks of one [128, 2, S] tile so a single ACT op
                # exps both heads' chunk; one bf16 DVE mul masks both diags
                mh = j
                exs = []
                for i in range(4):
                    lo = i * 128
                    sc = ps_b.tile([128, 2, S], f32, name="sc", tag="ps_b")
                    for hi, p0 in ((0, 0), (1, 64)):
                        nc.tensor.matmul(
                            sc[:, hi, 0:S - lo],
                            krot[mh][p0:p0 + 64, b * S + lo: b * S + lo + 128],
                            qrot[mh][p0:p0 + 64, b * S + lo: (b + 1) * S],
                            start=True, stop=True)
                    ex = expp.tile([128, 2, S], bf16, name="ex", tag=f"ex{i}")
                    nc.scalar.activation(ex[:, :, lo:S], sc[:, :, 0:S - lo], Exp, scale=0.125)
                    nc.vector.tensor_mul(ex[:, :, lo:lo + 128], ex[:, :, lo:lo + 128], mask2)
                    exs.append(ex)
                return exs

            def emit_pair_avden(b, j, exs):
                mh = j
                avs = []
                for hi in (0, 1):
                    h = 2 * j + hi
                    av = ps_a.tile([128, S], f32, name="av", tag="av", bufs=2)
                    for i in range(4):
                        lo = i * 128
                        nc.tensor.matmul(
                            av[0:65, lo:S],
                            vsb[b * 4 + i][:, h * 65: h * 65 + 65],
                            exs[i][:, hi, lo:S],
                            start=(i == 0), stop=(i == 3), skip_group_check=True)
                    avs.append(av)
                # softmax denominators: ACT copies both ones-rows to SBUF
                # (no external waits on the DVE queue), SBUF->SBUF DMAs
                # reshape them to [128,8] so one cheap DVE reciprocal covers
                # the pair, then a bf16 DRAM slot + stride-0 readback
                # broadcasts h0 to rows 0-63 and h1 to rows 64-127
                ss0 = work.tile([1, S], f32, name="ss0", tag="ss0", bufs=2)
                nc.scalar.copy(ss0, avs[0][64:65, :])
                ss1 = work.tile([1, S], f32, name="ss1", tag="ss1", bufs=2)
                nc.scalar.copy(ss1, avs[1][64:65, :])
                st8 = work.tile([128, 8], f32, name="st8", tag="st8", bufs=2)
                nc.gpsimd.dma_start(out=st8[:, 0:4], in_=ss0)
                nc.gpsimd.dma_start(out=st8[:, 4:8], in_=ss1)
                rt8 = work.tile([128, 8], bf16, name="rt8", tag="rt8", bufs=2)
                with nc.allow_low_precision(reason="softmax recip broadcast in bf16"):
                    nc.vector.reciprocal(rt8, st8)
                slot = dscr.tile([2, S], bf16, name="rr", tag="rr")
                nc.gpsimd.dma_start(
                    out=slot.rearrange("h (p f) -> p h f", f=4),
                    in_=rt8.rearrange("p (h f) -> p h f", f=4))
                rb = work.tile([128, S], bf16, name="rb", tag="rb", bufs=2)
                nc.gpsimd.dma_start(
                    out=rb, in_=slot[:, None, :].to_broadcast((2, 64, S)))
                return (b, j, avs, rb)

            def emit_pair_norm(st):
                # deferred one pair: by the time DVE reaches these, the rb
                # readback has long landed, so the queue never stalls on it
                b, j, avs, rb = st
                bcols = slice(b * S, (b + 1) * S)
                nc.vector.tensor_mul(att[j][0:64, bcols], avs[0][0:64, :], rb[0:64, :])
                nc.vector.tensor_mul(att[j][64:128, bcols], avs[1][0:64, :], rb[64:128, :])
                if dump_debug:
                    nc.sync.dma_start(out=rb_d[b * 8 + j], in_=rb)

            def emit_wo_group(b, m):
                bcols = slice(b * S, (b + 1) * S)
                fin = ps_a.tile([128, S], f32, name="fin", tag="acc", bufs=2)
                for k in range(KC):
                    nc.tensor.matmul(
                        fin, wo[k][:, m * 128:(m + 1) * 128], att[k][:, bcols],
                        start=(k == 0), stop=(k == KC - 1))
                ob = work.tile([128, S], bf16, name="ob", tag="ob", bufs=2)
                nc.vector.tensor_copy(ob, fin)
                nc.sync.dma_start(out=out_d[m * 128:(m + 1) * 128, bcols], in_=ob)

            # ---- schedule (baseline skeleton): bulk qk(b0) + v(b0) upfront
            # (dense DMA-paced warmup), then b0 pairs with qk(b1)/v(b1)
            # fillers after each pair, then b1 pairs with wo(b0) fillers,
            # then the wo(b1) tail.  Within a qk unit, B parts follow the
            # two A parts so the rp2 matmul's pre->pre2 chain hides behind
            # the second A's 8 matmuls.
            def emit_qk_unit(b, j):
                st_q = emit_qk_A(b, wq, qrot, j)
                st_k = emit_qk_A(b, wk, krot, j)
                emit_qk_B(st_q)
                emit_qk_B(st_k)

            for m in range(KC):
                emit_qk_unit(0, m)
            for nh in (0, 1):
                for tch in range(4):
                    emit_v_group(tch, nh)
            v1list = [(tch, nh) for nh in (0, 1) for tch in range(4, 8)]
            pend = None
            for j in range(H // 2):
                exs = emit_pair_scores(0, j)
                if pend is not None:
                    emit_pair_norm(pend)
                pend = emit_pair_avden(0, j, exs)
                emit_qk_unit(1, j)
                emit_v_group(*v1list[j])
            for j in range(H // 2):
                exs = emit_pair_scores(1, j)
                emit_pair_norm(pend)
                pend = emit_pair_avden(1, j, exs)
                emit_wo_group(0, j)
            emit_pair_norm(pend)
            for m in range(KC):
                emit_wo_group(1, m)

            if dump_debug:
                for m in range(KC):
                    rows = slice(m * 128, (m + 1) * 128)
                    nc.sync.dma_start(out=qrot_d[rows, :], in_=qrot[m])
                    nc.sync.dma_start(out=krot_d[rows, :], in_=krot[m])
                    nc.sync.dma_start(out=att_d[rows, :], in_=att[m])
                for t_ in range(T // 128):
                    nc.sync.dma_start(out=v_d[t_ * 128:(t_ + 1) * 128, :], in_=vsb[t_])

    nc.compile()
    return nc


def _get_nc():
    if "nc" not in _CACHE:
        _CACHE["nc"] = _build_bass()
    return _CACHE["nc"]


def make_in_maps(x, Wq, Wk, Wv, Wo):
    """Host-side shard + layout prep: one input dict per core."""
    cosT, tanT, RT, mask2 = _host_consts()
    shared = {
        "WqT": np.ascontiguousarray(Wq.T).astype(BF16),
        "WkT": np.ascontiguousarray(Wk.T).astype(BF16),
        "WvT": np.ascontiguousarray(Wv.T).astype(BF16),
        "WoT": np.ascontiguousarray(Wo.T).astype(BF16),
        "cosT": cosT,
        "tanT": tanT,
        "RT": RT,
        "mask2": mask2,
    }
    in_maps = []
    for c in range(NCORES):
        xc = x[c * BPC:(c + 1) * BPC]  # [BPC, S, D]
        xT = np.ascontiguousarray(xc.transpose(2, 0, 1).reshape(D, T)).astype(BF16)
        in_maps.append({"xT": xT, **shared})
    return in_maps


def assemble(results):
    """results: list (per core) of {"outT": [D, T] bf16} -> [B, S, D] fp32."""
    out = np.empty((B, S, D), np.float32)
    for c in range(NCORES):
        oT = np.asarray(results[c]["outT"]).astype(np.float32)
        out[c * BPC:(c + 1) * BPC] = oT.reshape(D, BPC, S).transpose(1, 2, 0)
    return out


def run(x, Wq, Wk, Wv, Wo, trace=False, **run_kwargs):
    from concourse.bass_utils import run_bass_kernel_spmd
    nc = _get_nc()
    in_maps = make_in_maps(x, Wq, Wk, Wv, Wo)
    res = run_bass_kernel_spmd(
        nc, in_maps, core_ids=list(range(NCORES)), trace=trace, **run_kwargs)
    return assemble(res.results), res


def kernel(x, Wq, Wk, Wv, Wo):
    out, _ = run(np.asarray(x), np.asarray(Wq), np.asarray(Wk),
                 np.asarray(Wv), np.asarray(Wo))
    return out


if __name__ == "__main__":
    rng = np.random.default_rng(0)
    scale = 1.0 / np.sqrt(D)
    inputs = {
        "x": rng.standard_normal((B, S, D), dtype=np.float32),
        "Wq": (rng.standard_normal((D, D), dtype=np.float32) * scale),
        "Wk": (rng.standard_normal((D, D), dtype=np.float32) * scale),
        "Wv": (rng.standard_normal((D, D), dtype=np.float32) * scale),
        "Wo": (rng.standard_normal((D, D), dtype=np.float32) * scale),
    }
    out = kernel(**inputs)
    print("out", out.shape, out.dtype, float(np.abs(out).max()))


# revision 46
# speedup vs baseline: 1.6964x; 1.6964x over previous
"""Multi-head attention (RoPE + causal softmax) Trainium2 Bass kernel.

Problem: nn_MultiHeadAttention (B=16, S=512, D=1024, H=16, Hd=64).
Sharding: data-parallel over batch — 2 batches per core on 8 NeuronCores.
Feature-major device layout; per-core pipeline: q/k projections + RoPE,
v projection (ones-column augmented), per-head-pair causal attention with
PE-packed score matmuls, softmax via exp + ones-row denominators, Wo
projection.  Measured ~244-246us on HW.
"""

import numpy as np
import ml_dtypes

BF16 = ml_dtypes.bfloat16

B, S, D = 16, 512, 1024
H, HD = 16, 64
NCORES = 8
BPC = B // NCORES
T = BPC * S

_CACHE = {}


def _rope_tables():
    inv_freq = 1.0 / (10000.0 ** (np.arange(0, HD, 2, dtype=np.float64) / HD))
    t = np.arange(S, dtype=np.float64)
    freqs = np.outer(t, inv_freq)
    emb = np.concatenate([freqs, freqs], -1)
    return np.cos(emb), np.sin(emb)


def _host_consts():
    cos, sin = _rope_tables()
    tan = sin / cos
    cols = np.arange(T) % S
    cosT = np.ascontiguousarray(np.tile(cos[cols].T, (2, 1))).astype(BF16)
    tanT = np.ascontiguousarray(np.tile(tan[cols].T, (2, 1))).astype(BF16)
    R64 = np.zeros((64, 64), np.float32)
    R64[np.arange(32), np.arange(32) + 32] = -1.0
    R64[np.arange(32) + 32, np.arange(32)] = 1.0
    R128 = np.zeros((128, 128), np.float32)
    R128[:64, :64] = R64
    R128[64:, 64:] = R64
    RT = np.ascontiguousarray(R128.T).astype(BF16)
    mask01 = (np.arange(128)[None, :] >= np.arange(128)[:, None]).astype(BF16)
    mask2 = np.ascontiguousarray(np.concatenate([mask01, mask01], axis=1))
    return cosT, tanT, RT, mask2


def _build_bass(dump_debug=False):
    import concourse.bacc as bacc
    import concourse.tile as tile
    import concourse.mybir as mybir

    dt = mybir.dt
    f32, bf16 = dt.float32, dt.bfloat16
    Exp = mybir.ActivationFunctionType.Exp

    nc = bacc.Bacc("TRN2", target_bir_lowering=False, debug=False, enable_asserts=False)

    xT_d = nc.dram_tensor("xT", [D, T], bf16, kind="ExternalInput").ap()
    wq_d = nc.dram_tensor("WqT", [D, D], bf16, kind="ExternalInput").ap()
    wk_d = nc.dram_tensor("WkT", [D, D], bf16, kind="ExternalInput").ap()
    wv_d = nc.dram_tensor("WvT", [D, D], bf16, kind="ExternalInput").ap()
    wo_d = nc.dram_tensor("WoT", [D, D], bf16, kind="ExternalInput").ap()
    cos_d = nc.dram_tensor("cosT", [128, T], bf16, kind="ExternalInput").ap()
    tan_d = nc.dram_tensor("tanT", [128, T], bf16, kind="ExternalInput").ap()
    rt_d = nc.dram_tensor("RT", [128, 128], bf16, kind="ExternalInput").ap()
    mask_d = nc.dram_tensor("mask2", [128, 256], bf16, kind="ExternalInput").ap()
    out_d = nc.dram_tensor("outT", [D, T], f32, kind="ExternalOutput").ap()

    KC = D // 128

    with tile.TileContext(nc) as tc:
        with (
            tc.tile_pool(name="consts", bufs=1) as consts,
            tc.tile_pool(name="persist", bufs=1) as persist,
            tc.tile_pool(name="work", bufs=3) as work,
            tc.tile_pool(name="expp", bufs=3) as expp,
            tc.tile_pool(name="ps_a", bufs=4, space="PSUM") as ps_a,
            tc.tile_pool(name="ps_b", bufs=2, space="PSUM") as ps_b,
        ):
            def load(pool, dram, shape, dtyp, nm):
                t_ = pool.tile(shape, dtyp, name=nm)
                nc.sync.dma_start(out=t_, in_=dram)
                return t_

            xT = [load(consts, xT_d[k * 128:(k + 1) * 128, :], [128, T], bf16, f"xT{k}") for k in range(KC)]
            wq = [load(consts, wq_d[k * 128:(k + 1) * 128, :], [128, D], bf16, f"wq{k}") for k in range(KC)]
            RT = load(consts, rt_d, [128, 128], bf16, "RT")
            cosT = load(consts, cos_d, [128, T], bf16, "cosT")
            tanT = load(consts, tan_d, [128, T], bf16, "tanT")
            wk = [load(consts, wk_d[k * 128:(k + 1) * 128, :], [128, D], bf16, f"wk{k}") for k in range(KC)]
            wv = [load(consts, wv_d[k * 128:(k + 1) * 128, :], [128, D], bf16, f"wv{k}") for k in range(KC)]
            mask2 = load(consts, mask_d, [128, 2, 128], bf16, "mask2")
            wo = [load(consts, wo_d[k * 128:(k + 1) * 128, :], [128, D], bf16, f"wo{k}") for k in range(KC)]

            qrot = [persist.tile([128, T], bf16, name=f"qrot{m}") for m in range(KC)]
            krot = [persist.tile([128, T], bf16, name=f"krot{m}") for m in range(KC)]
            vsb = [persist.tile([128, H * 65], bf16, name=f"vsb{t_}") for t_ in range(T // 128)]
            att = [persist.tile([128, T], bf16, name=f"att{m}") for m in range(KC)]

            for t_ in range(T // 128):
                vt = vsb[t_].rearrange("p (h w) -> p h w", w=65)
                nc.gpsimd.memset(vt[:, :, 64:65], 1.0)

            # qk projection with the RoPE tan trick, split into A (projection
            # matmuls + pre2 = pp*tan on DVE) and B (rotation matmul that
            # accumulates into the same PSUM + rot = pp*cos).  Emitting B
            # after the sibling A hides the pre2 chain behind 8 matmuls.
            # Identity: R@(pp*tan)*cos == rotate_half(pp)*sin because the
            # RoPE tables repeat with period 32.
            def emit_qk_A(nb, w_sb, rot, m):
                cols = slice(nb * S, (nb + 1) * S)
                pp = ps_a.tile([128, S], f32, name="pp", tag="ps_a")
                for k in range(KC):
                    nc.tensor.matmul(
                        pp, w_sb[k][:, m * 128:(m + 1) * 128], xT[k][:, cols],
                        start=(k == 0), stop=(k == KC - 1))
                pre2 = work.tile([128, S], bf16, name="pre2", tag="pre2", bufs=2)
                nc.vector.tensor_mul(pre2, pp, tanT[:, cols])
                return (pp, pre2, rot, m, cols)

            def emit_qk_B(st):
                pp, pre2, rot, m, cols = st
                nc.tensor.matmul(pp, RT, pre2, start=False, stop=True,
                                 skip_group_check=True)
                nc.vector.tensor_mul(rot[m][:, cols], pp, cosT[:, cols])

            def emit_qk_unit(nb, m):
                st_q = emit_qk_A(nb, wq, qrot, m)
                st_k = emit_qk_A(nb, wk, krot, m)
                emit_qk_B(st_q)
                emit_qk_B(st_k)

            def emit_v_group(b, tch, nh):
                vt = vsb[tch].rearrange("p (h w) -> p h w", w=65)
                vp = ps_a.tile([128, S], f32, name="vp", tag="ps_a")
                for k in range(KC):
                    nc.tensor.matmul(
                        vp, xT[k][:, tch * 128:(tch + 1) * 128],
                        wv[k][:, nh * S:(nh + 1) * S],
                        start=(k == 0), stop=(k == KC - 1))
                nc.scalar.copy(
                    vt[:, nh * 8:(nh + 1) * 8, 0:64],
                    vp.rearrange("p (h w) -> p h w", w=64))

            def emit_attn_head(b, h, exs):
                bcols = slice(b * S, (b + 1) * S)
                mh, p0 = h // 2, (h % 2) * 64
                hi = h % 2
                av = ps_a.tile([128, S], f32, name="av", tag="ps_a")
                for i in range(4):
                    lo = i * 128
                    nc.tensor.matmul(
                        av[0:65, lo:S],
                        vsb[b * 4 + i][:, h * 65: h * 65 + 65],
                        exs[i][:, hi, lo:S],
                        start=(i == 0), stop=(i == 3), skip_group_check=True)
                ss = work.tile([1, S], f32, name="ss", tag="ss")
                nc.vector.tensor_copy(ss, av[64:65, :])
                st = work.tile([128, 4], f32, name="st", tag="st")
                nc.gpsimd.dma_start(out=st, in_=ss)
                rt = work.tile([128, 4], f32, name="rt", tag="rt")
                nc.vector.reciprocal(rt, st)
                rr = work.tile([1, S], f32, name="rr", tag="rr")
                nc.gpsimd.dma_start(out=rr, in_=rt)
                rb = work.tile([64, S], f32, name="rb", tag="rb", bufs=2)
                nc.gpsimd.partition_broadcast(rb, rr)
                nc.vector.tensor_mul(att[mh][p0:p0 + 64, bcols], av[0:64, :], rb)

            def emit_attn_pair(b, j):
                mh = j
                exs = []
                for i in range(4):
                    lo = i * 128
                    sc = ps_b.tile([128, 2, S], f32, name="sc", tag="ps_b")
                    for hi, p0 in ((0, 0), (1, 64)):
                        nc.tensor.matmul(
                            sc[:, hi, 0:S - lo],
                            krot[mh][p0:p0 + 64, b * S + lo: b * S + lo + 128],
                            qrot[mh][p0:p0 + 64, b * S + lo: (b + 1) * S],
                            start=True, stop=True)
                    ex = expp.tile([128, 2, S], bf16, name="ex", tag=f"ex{i}")
                    nc.scalar.activation(ex[:, :, lo:S], sc[:, :, 0:S - lo], Exp, scale=0.125)
                    nc.vector.tensor_mul(ex[:, :, lo:lo + 128], ex[:, :, lo:lo + 128], mask2)
                    exs.append(ex)
                emit_attn_head(b, 2 * j, exs)
                emit_attn_head(b, 2 * j + 1, exs)

            def emit_wo_group(b, m):
                bcols = slice(b * S, (b + 1) * S)
                fin = ps_a.tile([128, S], f32, name="fin", tag="ps_a")
                for k in range(KC):
                    nc.tensor.matmul(
                        fin, wo[k][:, m * 128:(m + 1) * 128], att[k][:, bcols],
                        start=(k == 0), stop=(k == KC - 1))
                ob = work.tile([128, S], f32, name="ob", tag="ob", bufs=2)
                nc.vector.tensor_copy(ob, fin)
                nc.sync.dma_start(out=out_d[m * 128:(m + 1) * 128, bcols], in_=ob)

            for m in range(KC):
                emit_qk_unit(0, m)
            for tch in range(4):
                for nh in range(2):
                    emit_v_group(0, tch, nh)
            v1 = [(tch, nh) for tch in range(4, 8) for nh in range(2)]
            for j in range(H // 2):
                emit_attn_pair(0, j)
                emit_qk_unit(1, j)
                emit_v_group(1, *v1[j])
            for j in range(H // 2):
                emit_attn_pair(1, j)
                emit_wo_group(0, j)
            for m in range(KC):
                emit_wo_group(1, m)

    nc.compile()
    return nc


def _get_nc():
    if "nc" not in _CACHE:
        _CACHE["nc"] = _build_bass()
    return _CACHE["nc"]


def make_in_maps(x, Wq, Wk, Wv, Wo):
    cosT, tanT, RT, mask2 = _host_consts()
    shared = {
        "WqT": np.ascontiguousarray(Wq.T).astype(BF16),
        "WkT": np.ascontiguousarray(Wk.T).astype(BF16),
        "WvT": np.ascontiguousarray(Wv.T).astype(BF16),
        "WoT": np.ascontiguousarray(Wo.T).astype(BF16),
        "cosT": cosT,
        "tanT": tanT,
        "RT": RT,
        "mask2": mask2,
    }
    in_maps = []
    for c in range(NCORES):
        xc = x[c * BPC:(c + 1) * BPC]
        xT = np.ascontiguousarray(xc.transpose(2, 0, 1).reshape(D, T)).astype(BF16)
        in_maps.append({"xT": xT, **shared})
    return in_maps


def assemble(results):
    out = np.empty((B, S, D), np.float32)
    for c in range(NCORES):
        oT = np.asarray(results[c]["outT"])
        out[c * BPC:(c + 1) * BPC] = oT.reshape(D, BPC, S).transpose(1, 2, 0)
    return out


def run(x, Wq, Wk, Wv, Wo, trace=False, **run_kwargs):
    from concourse.bass_utils import run_bass_kernel_spmd
    nc = _get_nc()
    in_maps = make_in_maps(x, Wq, Wk, Wv, Wo)
    res = run_bass_kernel_spmd(
        nc, in_maps, core_ids=list(range(NCORES)), trace=trace, **run_kwargs)
    return assemble(res.results), res


def kernel(x, Wq, Wk, Wv, Wo):
    out, _ = run(np.asarray(x), np.asarray(Wq), np.asarray(Wk),
                 np.asarray(Wv), np.asarray(Wo))
    return out
